# revision 16
# baseline (speedup 1.0000x reference)
"""Trainium2 Bass kernel for the Disentangle out-layer GNN message-passing op.

    temp  = exp(Z @ Z.T)                 # [N, N]
    rowsum[i] = sum_j temp[i, j]
    alpha = temp / rowsum[None, :]       # column-indexed divisor
    p_adj = alpha * adj
    h     = BETA * Z + (1 - BETA) * (p_adj @ Z)
    returns (h, alpha)

temp is symmetric, so rowsum[j] == column-sum of column j.  We shard
*columns* j across the 8 cores and work in the transposed layout
tempT[j, i] (j on partitions, i on the free axis):
  - rowsum[j] is a local free-axis reduction (fused into the exp pass
    via activation accum_out),
  - the divide is a per-partition broadcast,
  - tempT (x) adjT serves directly as rhs for the h matmul, with the
    1/rowsum[j] folded into the per-strip stationary weights Zk*recip,
  - no cross-core collectives; the host sums the 8 hT partials.

bf16 everywhere except the fp32 PSUM accumulations and the rowsum;
adj is 0/1 so bf16 is exact.  alphaT is stored bf16 and upconverted on
the host.  Strips are processed in groups of G=2 so the h matmul
accumulates both strips inside one PSUM bank before the SBUF add.

Per-core outputs: alphaT strip [1024, 8192] bf16 and hT partial
[128, 8192] f32.  Host: alpha = concat(strips).T.astype(f32);
h = BETA*Z + (1-BETA)*sum(hT).T.
"""

import numpy as np

N = 8192
D = 128
P = 128
NCORES = 8
SHARD = N // NCORES          # 1024 columns j per core
BETA = 0.5

FB = 1024                    # i-window (2 PSUM banks)
MB = 512                     # matmul free-dim (1 PSUM bank)
DMAB = 2048                  # DMA chunk along i
G = 2                        # strips per h-accumulation group

_CACHE = {}


def _build(n=N, shard=SHARD):
    import concourse.bass as bass  # noqa: F401
    import concourse.tile as tile
    from concourse import bacc, mybir

    f32 = mybir.dt.float32
    bf16 = mybir.dt.bfloat16
    X = mybir.AxisListType.X
    MUL = mybir.AluOpType.mult
    ADD = mybir.AluOpType.add
    EXP = mybir.ActivationFunctionType.Exp

    tstrips = shard // P     # j-strips per core
    nfb = n // FB            # exp windows per strip
    ndb = n // DMAB          # DMA chunks per strip

    nc = bacc.Bacc(
        "TRN2", target_bir_lowering=False, debug=False, num_devices=NCORES
    )

    zt_d = nc.dram_tensor("zt", [D, n], bf16, kind="ExternalInput")       # Z.T full
    ztk_d = nc.dram_tensor("ztk", [D, shard], bf16, kind="ExternalInput")  # Z.T[:, cols_k]
    zk_d = nc.dram_tensor("zk", [shard, D], bf16, kind="ExternalInput")   # Z[cols_k, :]
    adj_d = nc.dram_tensor("adjt", [shard, n], bf16, kind="ExternalInput")  # adj[:, cols_k].T
    alpha_d = nc.dram_tensor("alphat", [shard, n], bf16, kind="ExternalOutput")
    ht_d = nc.dram_tensor("ht", [D, n], f32, kind="ExternalOutput")       # (p_adj_k @ Z_k).T

    with tile.TileContext(nc) as tc:
        with (
            tc.tile_pool(name="const", bufs=1) as cpool,
            tc.tile_pool(name="strip", bufs=2 * G) as spool,
            tc.tile_pool(name="adj", bufs=6) as apool,
            tc.tile_pool(name="alpha", bufs=4) as alpool,
            tc.tile_pool(name="padj", bufs=4) as ppool,
            tc.tile_pool(name="zkr", bufs=2 * G) as zkrpool,
            tc.tile_pool(name="rs", bufs=2 * G) as rpool,
            tc.tile_pool(name="ps_sc", bufs=2, space="PSUM") as ps_sc,
            tc.tile_pool(name="ps_h", bufs=2, space="PSUM") as ps_h,
        ):
            zts = [cpool.tile([P, n // 4], bf16, tag=f"zt{c}", name=f"zt{c}") for c in range(4)]
            ztk = cpool.tile([P, shard], bf16, tag="ztk")
            zk = cpool.tile([P, tstrips, D], bf16, tag="zk")
            ht = cpool.tile([P, n], f32, tag="ht")

            nc.sync.dma_start(ztk[:], ztk_d[:])
            for c in range(4):
                s = slice(c * (n // 4), (c + 1) * (n // 4))
                nc.sync.dma_start(zts[c][:], zt_d[:, s])
            nc.sync.dma_start(zk[:], zk_d.rearrange("(t p) d -> p t d", p=P))

            for g in range(tstrips // G):
                temps, recips, zkrs = [], [], []
                for tt in range(G):
                    t = g * G + tt
                    tj = slice(t * P, (t + 1) * P)
                    temp = spool.tile([P, n], bf16, tag="temp")
                    rsp = rpool.tile([P, nfb], f32, tag="rsp")
                    rs = rpool.tile([P, 1], f32, tag="rs")
                    recip = rpool.tile([P, 1], f32, tag="recip")

                    # scores (bf16 matmul, fp32 PSUM) + exp->bf16, fused rowsum
                    for w in range(nfb):
                        sc = ps_sc.tile([P, FB], f32, tag="sc")
                        for m in range(FB // MB):
                            lo = w * FB + m * MB
                            nc.tensor.matmul(
                                sc[:, m * MB : (m + 1) * MB],
                                ztk[:, tj],
                                zts[lo // (n // 4)][:, lo % (n // 4) : lo % (n // 4) + MB],
                                start=True,
                                stop=True,
                            )
                        nc.scalar.activation(
                            temp[:, w * FB : (w + 1) * FB],
                            sc[:],
                            EXP,
                            accum_out=rsp[:, w : w + 1],
                        )
                    nc.vector.reduce_sum(rs[:], rsp[:], axis=X)
                    nc.vector.reciprocal(recip[:], rs[:])

                    # fold 1/rowsum into the stationary h-matmul weights
                    zkr = zkrpool.tile([P, D], bf16, tag="zkr")
                    nc.vector.tensor_scalar_mul(zkr[:], zk[:, t, :], recip[:])
                    for c in range(ndb):
                        ci = slice(c * DMAB, (c + 1) * DMAB)
                        at = alpool.tile([P, DMAB], bf16, tag="at")
                        nc.vector.tensor_scalar_mul(at[:], temp[:, ci], recip[:])
                        nc.sync.dma_start(alpha_d[tj, ci], at[:])
                    temps.append(temp)
                    recips.append(recip)
                    zkrs.append(zkr)

                for c in range(ndb):
                    ci = slice(c * DMAB, (c + 1) * DMAB)
                    pas = []
                    for tt in range(G):
                        t = g * G + tt
                        tj = slice(t * P, (t + 1) * P)
                        aj = apool.tile([P, DMAB], bf16, tag="aj")
                        nc.sync.dma_start(aj[:], adj_d[tj, ci])
                        pa = ppool.tile([P, DMAB], bf16, tag="pa")
                        nc.vector.tensor_tensor(pa[:], temps[tt][:, ci], aj[:], MUL)
                        pas.append(pa)
                    for w in range(DMAB // FB):
                        gi = slice(c * DMAB + w * FB, c * DMAB + (w + 1) * FB)
                        hp = ps_h.tile([P, FB], f32, tag="hp")
                        for tt in range(G):
                            for m in range(FB // MB):
                                lo = w * FB + m * MB
                                nc.tensor.matmul(
                                    hp[:, m * MB : (m + 1) * MB],
                                    zkrs[tt],
                                    pas[tt][:, lo : lo + MB],
                                    start=(tt == 0),
                                    stop=(tt == G - 1),
                                )
                        if g == 0:
                            nc.scalar.copy(ht[:, gi], hp[:])
                        else:
                            nc.vector.tensor_tensor(ht[:, gi], ht[:, gi], hp[:], ADD)


            for c in range(ndb):
                s = slice(c * DMAB, (c + 1) * DMAB)
                nc.sync.dma_start(ht_d[:, s], ht[:, s])

    nc.compile()
    return nc


def _get_nc():
    if "nc" not in _CACHE:
        _CACHE["nc"] = _build()
    return _CACHE["nc"]


def _make_in_maps(Z, adj):
    import ml_dtypes

    bf16 = ml_dtypes.bfloat16
    ZTb = np.ascontiguousarray(Z.T).astype(bf16)          # [D, N]
    adjTb = np.ascontiguousarray(adj.T).astype(bf16)      # [N, N] (j, i)
    in_maps = []
    for k in range(NCORES):
        cols = slice(k * SHARD, (k + 1) * SHARD)
        in_maps.append(
            {
                "zt": ZTb,
                "ztk": np.ascontiguousarray(ZTb[:, cols]),
                "zk": Z[cols, :].astype(bf16),
                "adjt": adjTb[cols, :],
            }
        )
    return in_maps


def run_on_hw(Z, adj, trace=False, trace_cores=None):
    """Run the SPMD kernel on 8 cores; returns ((h, alpha), BassKernelResults)."""
    from concourse.bass_utils import run_bass_kernel_spmd

    Z = np.asarray(Z, dtype=np.float32)
    adj = np.asarray(adj, dtype=np.float32)
    assert Z.shape == (N, D) and adj.shape == (N, N)

    nc = _get_nc()
    in_maps = _make_in_maps(Z, adj)
    kw = {}
    if trace:
        kw = {"trace": True}
        if trace_cores is not None:
            kw["trace_cores"] = trace_cores
    res = run_bass_kernel_spmd(nc, in_maps, core_ids=list(range(NCORES)), **kw)

    alphaT = np.concatenate([r["alphat"] for r in res.results], axis=0)  # [N, N] (j, i) bf16
    alpha = np.ascontiguousarray(alphaT.T).astype(np.float32)
    ht = np.zeros((D, N), dtype=np.float32)
    for r in res.results:
        ht += r["ht"]
    h = BETA * Z + (1.0 - BETA) * np.ascontiguousarray(ht.T)
    return (h.astype(np.float32), alpha), res


def kernel(Z, adj):
    out, _ = run_on_hw(Z, adj, trace=False)
    return out
